# revision 5
# baseline (speedup 1.0000x reference)
"""Trainium2 Bass kernel for quantized int8 GEMM with fused dequant/requant epilogue.

Computes, for A:(M,K) int8-valued, B:(N,K) int8-valued (both stored int32):
    acc = A @ B^T                       (int32, exact)
    x   = acc * (scale_A/scale_out)[m] * scale_B[n]
    C   = clip(round_half_even(x), -128, 127).astype(int8)
returns (C, scale_out).

Strategy: shard A (and scale_A/scale_out) along M across 8 NeuronCores;
replicate B. Per core: cast operands to bf16 (exact for int8 values),
keep the A^T shard resident in SBUF, stream B^T in 8 n-slabs
(double-buffered), accumulate K=4096 in PSUM over 32 matmuls per
[128m x 512n] tile, then apply the whole epilogue as one fused DVE
scalar_tensor_tensor whose fp32->int8 output conversion is
round-half-even + saturating (verified on HW) - bit-exact vs the
jnp reference.
"""
import numpy as np
import ml_dtypes

M, K, N = 8192, 4096, 4096
NCORES = 8
MS = M // NCORES      # 1024 rows of A per core
KT = K // 128         # 32 k-tiles
NJ = N // 512         # 8 n-slabs
MI = MS // 128        # 8 m-tiles per core

_NC_CACHE = None
LAST_RESULTS = None   # BassKernelResults of the most recent run (for profiling)


def _build_nc():
    import concourse.bacc as bacc
    import concourse.mybir as mybir
    import concourse.tile as tile

    nc = bacc.Bacc("TRN2", target_bir_lowering=False, debug=False,
                   enable_asserts=False)
    # host-prepped layouts, all per-partition-contiguous for DMA:
    #   a_t[p, t, m]    = A[m_off+m, t*128+p]          (A^T shard, bf16)
    #   b_t[p, j, t, n] = B[j*512+n, t*128+p]          (B^T, bf16)
    #   s[p, mi]        = (scale_A/scale_out)[m_off + mi*128+p]
    #   sb[p, n]        = scale_B[n]                    (broadcast over p)
    #   c[p, j, mi, n]  = C[m_off + mi*128+p, j*512+n]
    a_t = nc.dram_tensor("a_t", (128, KT, MS), mybir.dt.bfloat16, kind="ExternalInput")
    b_t = nc.dram_tensor("b_t", (128, NJ, KT, 512), mybir.dt.bfloat16, kind="ExternalInput")
    s = nc.dram_tensor("s", (128, MI), mybir.dt.float32, kind="ExternalInput")
    sb = nc.dram_tensor("sb", (128, N), mybir.dt.float32, kind="ExternalInput")
    c = nc.dram_tensor("c", (128, NJ, MI, 512), mybir.dt.int8, kind="ExternalOutput")

    with tile.TileContext(nc) as tc:
        with (
            tc.tile_pool(name="a", bufs=1) as apool,
            tc.tile_pool(name="b", bufs=2) as bpool,
            tc.tile_pool(name="sc", bufs=1) as spool,
            tc.tile_pool(name="ps", bufs=8, space="PSUM") as pspool,
            tc.tile_pool(name="o", bufs=2) as opool,
        ):
            s_sb = spool.tile([128, MI], mybir.dt.float32)
            nc.sync.dma_start(s_sb[:], s.ap())
            sb_sb = spool.tile([128, N], mybir.dt.float32)
            nc.sync.dma_start(sb_sb[:], sb.ap())
            # slab-0 B and resident A stream in as interleaved per-k-tile
            # chunks, in k order, so the k-outer slab-0 compute can start
            # as soon as chunk 0 lands instead of after the full 12.6 MB
            a_res = apool.tile([128, KT, MS], mybir.dt.bfloat16)
            b0 = bpool.tile([128, KT, 512], mybir.dt.bfloat16, tag="bslab")
            for t in range(KT):
                nc.sync.dma_start(b0[:, t], b_t.ap()[:, 0, t])
                nc.sync.dma_start(a_res[:, t], a_t.ap()[:, t])

            for j in range(NJ):
                if j == 0:
                    b_slab = b0
                else:
                    b_slab = bpool.tile([128, KT, 512], mybir.dt.bfloat16,
                                        tag="bslab")
                    nc.sync.dma_start(b_slab[:], b_t.ap()[:, j])
                o_sb = opool.tile([128, MI, 512], mybir.dt.int8, tag="osb")
                if j == 0:
                    # k-outer: all 8 m-tiles accumulate in parallel across the
                    # 8 PSUM banks; each k step consumes exactly chunk k
                    ps_tiles = []
                    for _mi in range(MI):
                        ps = pspool.tile([128, 512], mybir.dt.float32, tag="ps")
                        ps_tiles.append(ps)
                    for k in range(KT):
                        for mi in range(MI):
                            nc.tensor.matmul(
                                ps_tiles[mi][:],
                                lhsT=a_res[:, k, mi * 128:(mi + 1) * 128],
                                rhs=b_slab[:, k, :],
                                start=(k == 0), stop=(k == KT - 1),
                            )
                    for mi in range(MI):
                        nc.vector.scalar_tensor_tensor(
                            o_sb[:, mi, :], ps_tiles[mi][:], s_sb[:, mi:mi + 1],
                            sb_sb[:, j * 512:(j + 1) * 512],
                            op0=mybir.AluOpType.mult, op1=mybir.AluOpType.mult,
                        )
                else:
                    # mi-outer: B slab fully prefetched, fixed PSUM bank per
                    # 32-matmul accumulation group
                    for mi in range(MI):
                        ps = pspool.tile([128, 512], mybir.dt.float32, tag="ps")
                        for k in range(KT):
                            nc.tensor.matmul(
                                ps[:],
                                lhsT=a_res[:, k, mi * 128:(mi + 1) * 128],
                                rhs=b_slab[:, k, :],
                                start=(k == 0), stop=(k == KT - 1),
                            )
                        # full epilogue in one DVE op; fp32->int8 output cast
                        # is round-half-even + saturating
                        nc.vector.scalar_tensor_tensor(
                            o_sb[:, mi, :], ps[:], s_sb[:, mi:mi + 1],
                            sb_sb[:, j * 512:(j + 1) * 512],
                            op0=mybir.AluOpType.mult, op1=mybir.AluOpType.mult,
                        )
                nc.sync.dma_start(c.ap()[:, j], o_sb[:])

    nc.compile()
    return nc


def _get_nc():
    global _NC_CACHE
    if _NC_CACHE is None:
        _NC_CACHE = _build_nc()
    return _NC_CACHE


def prepare_in_maps(A, scale_A, B, scale_B, scale_out):
    """Host-side shard/layout prep -> per-core input maps."""
    bf16 = ml_dtypes.bfloat16
    A = np.asarray(A)
    B = np.asarray(B)
    scale_A = np.asarray(scale_A, dtype=np.float32)
    scale_B = np.asarray(scale_B, dtype=np.float32)
    scale_out = np.asarray(scale_out, dtype=np.float32)

    # B^T in slab-major layout: [p, j, t, n]
    b_host = np.ascontiguousarray(
        B.astype(np.float32).astype(bf16).reshape(NJ, 512, KT, 128)
        .transpose(3, 0, 2, 1))
    sb_host = np.ascontiguousarray(
        np.broadcast_to(scale_B[None, :], (128, N)))
    # compute scale_A/scale_out with the same device lowering the jax
    # reference uses (neuron divide is reciprocal-based, ~1ulp off IEEE on
    # ~28% of elements; using the same quotient keeps round-to-nearest ties
    # bit-identical to the reference)
    import jax.numpy as jnp
    s_full = np.asarray(jnp.asarray(scale_A) / jnp.asarray(scale_out),
                        dtype=np.float32)

    in_maps = []
    for cidx in range(NCORES):
        a_shard = A[cidx * MS:(cidx + 1) * MS]  # (MS, K)
        a_host = np.ascontiguousarray(
            a_shard.astype(np.float32).astype(bf16).reshape(MS, KT, 128)
            .transpose(2, 1, 0))
        s_host = np.ascontiguousarray(
            s_full[cidx * MS:(cidx + 1) * MS].reshape(MI, 128).T)
        in_maps.append({"a_t": a_host, "b_t": b_host, "s": s_host, "sb": sb_host})
    return in_maps


def assemble_output(per_core_results, scale_out):
    shards = []
    for cidx in range(NCORES):
        c_np = per_core_results[cidx]["c"]  # (128, NJ, MI, 512) int8
        shards.append(c_np.transpose(2, 0, 1, 3).reshape(MS, N))
    C = np.concatenate(shards, axis=0)
    return C.astype(np.int8), np.asarray(scale_out, dtype=np.float32)


def kernel(A, scale_A, B, scale_B, scale_out, _trace=False):
    global LAST_RESULTS
    from concourse.bass_utils import run_bass_kernel_spmd

    in_maps = prepare_in_maps(A, scale_A, B, scale_B, scale_out)
    nc = _get_nc()
    res = run_bass_kernel_spmd(nc, in_maps, core_ids=list(range(NCORES)),
                               trace=_trace)
    LAST_RESULTS = res
    return assemble_output(res.results, scale_out)


# revision 7
# speedup vs baseline: 1.3896x; 1.3896x over previous
"""Trainium2 Bass kernel for quantized int8 GEMM with fused dequant/requant epilogue.

Computes, for A:(M,K) int8-valued, B:(N,K) int8-valued (both stored int32):
    acc = A @ B^T                       (int32, exact)
    x   = acc * (scale_A/scale_out)[m] * scale_B[n]
    C   = clip(round_half_even(x), -128, 127).astype(int8)
returns (C, scale_out).

Strategy: shard A (and scale_A/scale_out) along M across 8 NeuronCores;
replicate B. Per core: cast operands to bf16 (exact for int8 values),
keep the A^T shard resident in SBUF, stream B^T in 8 n-slabs
(double-buffered), accumulate K=4096 in PSUM over 32 matmuls per
[128m x 512n] tile, then apply the whole epilogue as one fused DVE
scalar_tensor_tensor whose fp32->int8 output conversion is
round-half-even + saturating (verified on HW) - bit-exact vs the
jnp reference.
"""
import numpy as np
import ml_dtypes

M, K, N = 8192, 4096, 4096
NCORES = 8
MS = M // NCORES      # 1024 rows of A per core
KT = K // 128         # 32 k-tiles
NJ = N // 512         # 8 n-slabs
MI = MS // 128        # 8 m-tiles per core

_NC_CACHE = None
LAST_RESULTS = None   # BassKernelResults of the most recent run (for profiling)


def _build_nc():
    import concourse.bacc as bacc
    import concourse.mybir as mybir
    import concourse.tile as tile

    nc = bacc.Bacc("TRN2", target_bir_lowering=False, debug=False,
                   enable_asserts=False)
    # host-prepped layouts, all per-partition-contiguous for DMA:
    #   a_t[p, t, m]    = A[m_off+m, t*128+p]          (A^T shard, bf16)
    #   b_t[p, j, t, n] = B[j*512+n, t*128+p]          (B^T, bf16)
    #   s[p, mi]        = (scale_A/scale_out)[m_off + mi*128+p]
    #   sb[p, n]        = scale_B[n]                    (broadcast over p)
    #   c[p, j, mi, n]  = C[m_off + mi*128+p, j*512+n]
    a_t = nc.dram_tensor("a_t", (128, KT, MS), mybir.dt.bfloat16, kind="ExternalInput")
    b_t = nc.dram_tensor("b_t", (128, NJ, KT, 512), mybir.dt.bfloat16, kind="ExternalInput")
    s = nc.dram_tensor("s", (128, MI), mybir.dt.float32, kind="ExternalInput")
    sb = nc.dram_tensor("sb", (128, N), mybir.dt.float32, kind="ExternalInput")
    c = nc.dram_tensor("c", (128, NJ, MI, 512), mybir.dt.int8, kind="ExternalOutput")

    with tile.TileContext(nc) as tc:
        with (
            tc.tile_pool(name="a", bufs=1) as apool,
            tc.tile_pool(name="b", bufs=2) as bpool,
            tc.tile_pool(name="sc", bufs=1) as spool,
            tc.tile_pool(name="ps", bufs=8, space="PSUM") as pspool,
            tc.tile_pool(name="o", bufs=2) as opool,
        ):
            s_sb = spool.tile([128, MI], mybir.dt.float32)
            nc.sync.dma_start(s_sb[:], s.ap())
            sb_sb = spool.tile([128, N], mybir.dt.float32)
            nc.sync.dma_start(sb_sb[:], sb.ap())
            # slab-0 B and resident A stream in as interleaved per-k-tile
            # chunks, in k order, so the k-outer slab-0 compute can start
            # as soon as chunk 0 lands instead of after the full 12.6 MB
            a_res = apool.tile([128, KT, MS], mybir.dt.bfloat16)
            b0 = bpool.tile([128, KT, 512], mybir.dt.bfloat16, tag="bslab")
            for t in range(KT):
                nc.sync.dma_start(b0[:, t], b_t.ap()[:, 0, t])
                nc.sync.dma_start(a_res[:, t], a_t.ap()[:, t])

            for j in range(NJ):
                if j == 0:
                    b_slab = b0
                else:
                    b_slab = bpool.tile([128, KT, 512], mybir.dt.bfloat16,
                                        tag="bslab")
                    nc.sync.dma_start(b_slab[:], b_t.ap()[:, j])
                o_sb = opool.tile([128, MI, 512], mybir.dt.int8, tag="osb")
                if j == 0:
                    # k-outer: all 8 m-tiles accumulate in parallel across the
                    # 8 PSUM banks; each k step consumes exactly chunk k
                    ps_tiles = []
                    for _mi in range(MI):
                        ps = pspool.tile([128, 512], mybir.dt.float32, tag="ps")
                        ps_tiles.append(ps)
                    for k in range(KT):
                        for mi in range(MI):
                            nc.tensor.matmul(
                                ps_tiles[mi][:],
                                lhsT=a_res[:, k, mi * 128:(mi + 1) * 128],
                                rhs=b_slab[:, k, :],
                                start=(k == 0), stop=(k == KT - 1),
                            )
                    for mi in range(MI):
                        nc.vector.scalar_tensor_tensor(
                            o_sb[:, mi, :], ps_tiles[mi][:], s_sb[:, mi:mi + 1],
                            sb_sb[:, j * 512:(j + 1) * 512],
                            op0=mybir.AluOpType.mult, op1=mybir.AluOpType.mult,
                        )
                else:
                    # mi-outer: B slab fully prefetched, fixed PSUM bank per
                    # 32-matmul accumulation group
                    for mi in range(MI):
                        ps = pspool.tile([128, 512], mybir.dt.float32, tag="ps")
                        for k in range(KT):
                            nc.tensor.matmul(
                                ps[:],
                                lhsT=a_res[:, k, mi * 128:(mi + 1) * 128],
                                rhs=b_slab[:, k, :],
                                start=(k == 0), stop=(k == KT - 1),
                            )
                        # full epilogue in one DVE op; fp32->int8 output cast
                        # is round-half-even + saturating
                        nc.vector.scalar_tensor_tensor(
                            o_sb[:, mi, :], ps[:], s_sb[:, mi:mi + 1],
                            sb_sb[:, j * 512:(j + 1) * 512],
                            op0=mybir.AluOpType.mult, op1=mybir.AluOpType.mult,
                        )
                        if j == NJ - 1:
                            # last slab: drain each m-tile as soon as its
                            # epilogue lands to shorten the kernel tail
                            nc.sync.dma_start(c.ap()[:, j, mi], o_sb[:, mi, :])
                if j < NJ - 1:
                    nc.sync.dma_start(c.ap()[:, j], o_sb[:])

    nc.compile()
    return nc


def _get_nc():
    global _NC_CACHE
    if _NC_CACHE is None:
        _NC_CACHE = _build_nc()
    return _NC_CACHE


def prepare_in_maps(A, scale_A, B, scale_B, scale_out):
    """Host-side shard/layout prep -> per-core input maps."""
    bf16 = ml_dtypes.bfloat16
    A = np.asarray(A)
    B = np.asarray(B)
    scale_A = np.asarray(scale_A, dtype=np.float32)
    scale_B = np.asarray(scale_B, dtype=np.float32)
    scale_out = np.asarray(scale_out, dtype=np.float32)

    # B^T in slab-major layout: [p, j, t, n]
    b_host = np.ascontiguousarray(
        B.astype(np.float32).astype(bf16).reshape(NJ, 512, KT, 128)
        .transpose(3, 0, 2, 1))
    sb_host = np.ascontiguousarray(
        np.broadcast_to(scale_B[None, :], (128, N)))
    # compute scale_A/scale_out with the same device lowering the jax
    # reference uses (neuron divide is reciprocal-based, ~1ulp off IEEE on
    # ~28% of elements; using the same quotient keeps round-to-nearest ties
    # bit-identical to the reference)
    import jax.numpy as jnp
    s_full = np.asarray(jnp.asarray(scale_A) / jnp.asarray(scale_out),
                        dtype=np.float32)

    in_maps = []
    for cidx in range(NCORES):
        a_shard = A[cidx * MS:(cidx + 1) * MS]  # (MS, K)
        a_host = np.ascontiguousarray(
            a_shard.astype(np.float32).astype(bf16).reshape(MS, KT, 128)
            .transpose(2, 1, 0))
        s_host = np.ascontiguousarray(
            s_full[cidx * MS:(cidx + 1) * MS].reshape(MI, 128).T)
        in_maps.append({"a_t": a_host, "b_t": b_host, "s": s_host, "sb": sb_host})
    return in_maps


def assemble_output(per_core_results, scale_out):
    shards = []
    for cidx in range(NCORES):
        c_np = per_core_results[cidx]["c"]  # (128, NJ, MI, 512) int8
        shards.append(c_np.transpose(2, 0, 1, 3).reshape(MS, N))
    C = np.concatenate(shards, axis=0)
    return C.astype(np.int8), np.asarray(scale_out, dtype=np.float32)


def kernel(A, scale_A, B, scale_B, scale_out, _trace=False):
    global LAST_RESULTS
    from concourse.bass_utils import run_bass_kernel_spmd

    in_maps = prepare_in_maps(A, scale_A, B, scale_B, scale_out)
    nc = _get_nc()
    try:
        res = run_bass_kernel_spmd(nc, in_maps, core_ids=list(range(NCORES)),
                                   trace=_trace)
    except ModuleNotFoundError:
        # trace path requested (e.g. BASS_TRACE=1) but the axon NTFF hook
        # isn't available in this container - rerun without tracing
        import os
        os.environ["BASS_NEVER_TRACE"] = "1"
        res = run_bass_kernel_spmd(nc, in_maps, core_ids=list(range(NCORES)))
    LAST_RESULTS = res
    return assemble_output(res.results, scale_out)
